# revision 2
# baseline (speedup 1.0000x reference)
"""LAINet forward (nn_LAINetOriginal) as a Bass/Tile kernel on 8 trn2 cores.

Window sharding 8 x 125 (+37-window reflect halo recomputed per core so the
conv smoother is core-local). Per core: bf16 matmul1 with b1 folded as an
extra contraction row, fused ReLU copy, batch-norm via single bn_stats per
4-window group, block-diagonal W2 matmul2, free-axis softmax, PE transposes
with host-built permutation gathers, conv as Toeplitz-band matmuls with
shared-psum channel-reflect accumulation.

Self-contained: host shards full inputs, compiles + runs the SPMD NEFF via
concourse/bass (system toolchain), gathers full outputs. Numpy fallback if
the device path is unavailable.
"""
import sys

for _p in ('/opt/trn_rl_repo',):
    if _p not in sys.path:
        sys.path.insert(0, _p)

"""LAINet forward on trn2 — Bass/Tile kernel builder + host shard prep.

Sharding: 1000 windows split 8 x 125; each core recomputes a 37-window
reflect-mapped halo so the conv smoother is fully core-local.

Per-core layout (bc = b*2 + c, channel-interleaved):
  matmul1: per window n (200 = 125 own + 74 halo + 1 pad), 4 K-chunks of 125:
    lhsT = W1 chunk [125, 32(h padded)], rhs = x chunk [125, 128(bc)]
    -> psum1 [128, 128], 4 windows packed at 32-partition strides.
  relu+b1 fused in ACT copy psum->sbuf (bf16).
  BN over b: single bn_stats per group (even/odd lanes = c0/c1), variance
    read straight from the count*var slot with scale=1/64; apply on GpSimd.
  matmul2: lhsT = h_norm group [128, 128], rhs = block-diag W2 [128, 28]
    -> psum2 [128, bank, slot*28] ([bc, n, a] layout).
  softmax over a on the free axis; p -> PE transposes -> [win, a, bc].
  conv: Toeplitz-band matmuls, lhsT = pT/pS (pair-swapped) with a c-major
    column reorder so slot o accumulates both x-taps in shared psum
    partitions: m<64 -> out[cx=1], m>=64 -> out[cx=0] (host flips cx).
"""
import numpy as np
import ml_dtypes

BF16NP = ml_dtypes.bfloat16
FP8NP = ml_dtypes.float8_e4m3
W1SCALE = 16.0

B = 64
INPUT_DIM = 500000
WIN = 500
N_WIN = 1000
HID = 30
HP = 32          # padded hidden
ANC = 7
KS = 75
EPS = 1e-5
NCORES = 8
OWN = 125
HALO = 37
LW = 199
LWP = 200        # padded local window count
NG = 50          # groups of 4 windows
GPB = (18, 18, 14)   # groups per psum2 bank
NWB = (72, 72, 56)   # windows per bank
Y0S = (0, 54, 70, 97)    # conv block output offsets
RS = (128, 90, 101, 102) # conv block input row counts
NOUTS = (54, 16, 27, 28) # conv block output counts
GBAT = 5             # groups per DMA batch
NBAT = NG // GBAT    # 10


def build(nc, conv_b, swap_combine=True):
    """Construct the SPMD per-core program. conv_b: np [7] baked as imms."""
    import concourse.tile as tile
    import concourse.mybir as mybir
    from contextlib import ExitStack

    F32 = mybir.dt.float32
    BF = mybir.dt.bfloat16
    AF = mybir.ActivationFunctionType
    OP = mybir.AluOpType
    AX = mybir.AxisListType

    xk = nc.dram_tensor("xk", [25, 128, 8, 4, 128], BF,
                        kind="ExternalInput")
    w1k = nc.dram_tensor("w1k", [25, 128, 8, 4, HP], BF,
                         kind="ExternalInput")
    w2k = nc.dram_tensor("w2k", [128, 3, 504], BF, kind="ExternalInput")
    b2k = nc.dram_tensor("b2k", [128, 3, 504], F32, kind="ExternalInput")
    tpzk = nc.dram_tensor("tpzk", [128, ANC, 756], BF, kind="ExternalInput")
    prmk = nc.dram_tensor("prmk", [128, 2, 128], BF, kind="ExternalInput")
    ob_d = nc.dram_tensor("ob", [128, 875], F32, kind="ExternalOutput")
    sm_d = nc.dram_tensor("sm", [128, 875], F32, kind="ExternalOutput")

    with tile.TileContext(nc) as tc, ExitStack() as ctx:
        const = ctx.enter_context(tc.tile_pool(name="const", bufs=1))
        o_sb = const.tile([128, 3, 504], F32)
        e_sb = const.tile([128, 3, 504], BF)
        p_sb = const.tile([128, LWP, ANC], BF)
        sm_sb = const.tile([128, 125, ANC], F32)

        xp = ctx.enter_context(tc.tile_pool(name="xp", bufs=8))
        wp = ctx.enter_context(tc.tile_pool(name="wp", bufs=8))
        hpool = ctx.enter_context(tc.tile_pool(name="hp", bufs=6))
        sp = ctx.enter_context(tc.tile_pool(name="sp", bufs=3))
        ptp = ctx.enter_context(tc.tile_pool(name="ptp", bufs=2))
        ps1 = ctx.enter_context(tc.tile_pool(name="ps1", bufs=3, space="PSUM"))
        ps2 = ctx.enter_context(tc.tile_pool(name="ps2", bufs=2, space="PSUM"))
        tpq = ctx.enter_context(tc.tile_pool(name="tpq", bufs=2, space="PSUM"))
        pcp = ctx.enter_context(tc.tile_pool(name="pcp", bufs=1, space="PSUM"))

        # pair-sized (2-group) transfers; x on the sync HWDGE ring,
        # w1 via gpsimd SWDGE, consts on the scalar ring.
        xts = []
        wts = []
        for p in range(25):
            xt = xp.tile([128, 8, 4, 128], BF, tag="xb", name=f"xt{p}")
            nc.sync.dma_start(out=xt, in_=xk.ap()[p])
            wt = wp.tile([128, 8, 4, HP], BF, tag="wb", name=f"wt{p}")
            nc.gpsimd.dma_start(out=wt, in_=w1k.ap()[p])
            xts.append(xt)
            wts.append(wt)
            if p == 0:
                w2_sb = const.tile([128, 3, 504], BF)
                nc.scalar.dma_start(out=w2_sb, in_=w2k.ap())
                b2_sb = const.tile([128, 3, 504], F32)
                nc.scalar.dma_start(out=b2_sb, in_=b2k.ap())
                tz_sb = const.tile([128, ANC, 756], BF)
                nc.scalar.dma_start(out=tz_sb, in_=tpzk.ap())
                prm_sb = const.tile([128, 2, 128], BF)
                nc.scalar.dma_start(out=prm_sb, in_=prmk.ap())
                eps_sb = const.tile([128, 1], F32)
                nc.vector.memset(eps_sb, EPS)

        psum2 = {}

        def softmax_bank(j):
            nwj = NWB[j]
            cols = nwj * ANC
            nc.vector.tensor_tensor(
                out=o_sb[:, j, 0:cols], in0=psum2[j][:, 0:cols],
                in1=b2_sb[:, j, 0:cols], op=OP.add)
            nc.scalar.activation(out=e_sb[:, j, 0:cols],
                                 in_=o_sb[:, j, 0:cols], func=AF.Exp)
            ev = e_sb[:, j, 0:cols].rearrange("p (w a) -> p w a", a=ANC)
            rsum = sp.tile([128, 72], F32, tag="rsum", name=f"rsum{j}")
            nc.vector.reduce_sum(out=rsum[:, 0:nwj], in_=ev, axis=AX.X)
            nc.vector.reciprocal(out=rsum[:, 0:nwj], in_=rsum[:, 0:nwj])
            rb = rsum[:, 0:nwj].broadcast_to([128, nwj, ANC])
            nc.vector.tensor_tensor(
                out=p_sb[:, 72 * j:72 * j + nwj, :], in0=ev, in1=rb,
                op=OP.mult)
            # out_base slice for this bank (own windows 37..161)
            lo, hi = max(0, 37 - 72 * j), min(nwj, 162 - 72 * j)
            off = (72 * j + lo) - 37
            nc.gpsimd.dma_start(
                out=ob_d.ap()[:, 7 * off:7 * (off + hi - lo)],
                in_=o_sb[:, j, 7 * lo:7 * hi])

        def conv_block(t):
            y0, r, nout = Y0S[t], RS[t], NOUTS[t]
            pT = ptp.tile([128, ANC, 128], BF, tag="pT", name=f"pT{t}")
            pS = ptp.tile([128, ANC, 128], BF, tag="pS", name=f"pS{t}")
            for a2 in range(ANC):
                tp = tpq.tile([128, 128], BF, tag="tpq", name=f"tp{t}_{a2}")
                nc.tensor.transpose(out=tp[0:r, :], in_=p_sb[:, y0:y0 + r, a2],
                                    identity=prm_sb[:, 0, :])
                if a2 % 2 == 0:
                    nc.vector.tensor_copy(out=pT[0:r, a2, :], in_=tp[0:r, :])
                else:
                    nc.scalar.copy(out=pT[0:r, a2, :], in_=tp[0:r, :])
                tq = tpq.tile([128, 128], BF, tag="tpq", name=f"tq{t}_{a2}")
                nc.tensor.transpose(out=tq[0:r, :], in_=p_sb[:, y0:y0 + r, a2],
                                    identity=prm_sb[:, 1, :])
                if a2 % 2 == 0:
                    nc.scalar.copy(out=pS[0:r, a2, :], in_=tq[0:r, :])
                else:
                    nc.vector.tensor_copy(out=pS[0:r, a2, :], in_=tq[0:r, :])
            psc = pcp.tile([128, 378], F32, tag="psc", name=f"psc{t}")
            for i in range(ANC):
                nc.tensor.matmul(psc, lhsT=pT[0:r, i, :],
                                 rhs=tz_sb[0:r, i, 0:378],
                                 start=(i == 0), stop=False)
            for i in range(ANC):
                nc.tensor.matmul(psc, lhsT=pS[0:r, i, :],
                                 rhs=tz_sb[0:r, i, 378:756],
                                 start=False, stop=(i == 6))
            for o in range(ANC):
                nc.vector.tensor_scalar_add(
                    out=sm_sb[:, y0:y0 + nout, o],
                    in0=psc[:, 54 * o:54 * o + nout],
                    scalar1=float(conv_b[o]))
            nc.gpsimd.dma_start(
                out=sm_d.ap()[:, 7 * y0:7 * (y0 + nout)],
                in_=sm_sb[:, y0:y0 + nout, :])


        # pair compute: mm1 x32 -> relu -> stats; batch (5-group) sqrt,
        # then applies + mm2 per group; softmax as each bank completes.
        sts = {}
        hgs = {}
        for p in range(25):
            pt = ps1.tile([128, 256], F32, tag="pt", name=f"pt{p}")
            for sl in range(2):
                g = 2 * p + sl
                for k in range(4):
                    for ch in range(4):
                        kk = 126 if ch == 0 else 125
                        nc.tensor.matmul(
                            pt[32 * k:32 * k + HP, 128 * sl:128 * sl + 128],
                            lhsT=wts[p][0:kk, 4 * sl + k, ch, :],
                            rhs=xts[p][0:kk, 4 * sl + k, ch, :],
                            start=(ch == 0), stop=(ch == 3),
                            tile_position=(0, 32 * k))
            hg = hpool.tile([128, 256], BF, tag="hg", name=f"hg{p}")
            nc.scalar.activation(
                out=hg, in_=pt, func=AF.Relu, bias=0.0, scale=1.0)
            for sl in range(2):
                g = 2 * p + sl
                ib = g // GBAT
                if ib not in sts:
                    sts[ib] = sp.tile([128, GBAT, 6], F32, tag="st",
                                      name=f"st{ib}")
                hgs[g] = (hg, sl)
                nc.vector.bn_stats(out=sts[ib][:, g % GBAT, :],
                                   in_=hg[:, 128 * sl:128 * sl + 128])
                if g % GBAT != GBAT - 1:
                    continue
                st = sts[ib]
                stv = st.rearrange("p g (c s) -> p g c s", s=3)
                rs = sp.tile([128, GBAT, 2], F32, tag="rs", name=f"rs{ib}")
                nc.scalar.activation(
                    out=rs, in_=stv[:, :, :, 2], func=AF.Sqrt,
                    bias=eps_sb[:, 0:1], scale=1.0 / 64.0)
                nc.vector.reciprocal(out=rs, in_=rs)
                for gg in range(GBAT):
                    g2 = GBAT * ib + gg
                    hg2, sl2 = hgs[g2]
                    hgi = hg2.rearrange("p (s b c) -> p s b c", b=64, c=2)
                    for c in range(2):
                        nc.vector.tensor_scalar(
                            out=hgi[:, sl2, :, c], in0=hgi[:, sl2, :, c],
                            scalar1=stv[:, gg, c, 1:2],
                            scalar2=rs[:, gg, c:c + 1],
                            op0=OP.subtract, op1=OP.mult)
                    j = g2 // 18
                    sj = g2 - 18 * j
                    if j not in psum2:
                        psum2[j] = ps2.tile([128, 512], F32, tag="ps2",
                                            name=f"psum2_{j}")
                    nc.tensor.matmul(
                        psum2[j][:, 28 * sj:28 * sj + 28],
                        lhsT=hg2[:, 128 * sl2:128 * sl2 + 128],
                        rhs=w2_sb[:, j, 28 * sj:28 * sj + 28],
                        start=True, stop=True)
                for j in range(3):
                    if min(18 * j + 17, NG - 1) // GBAT == ib:
                        softmax_bank(j)

        # conv: transposes + Toeplitz matmuls with shared-slot accumulation.
        # A-branch (xt=0) uses pT (c-major gather); B-branch (xt=1) uses the
        # channel-flipped gather pS: slot o partition m holds out[cx=1] for
        # m<64 and out[cx=0] for m>=64 (host flips cx).
        for t in range(len(Y0S)):
            conv_block(t)
        return nc


def core_windows(k):
    ids = []
    for i in range(OWN * k - HALO, OWN * (k + 1) + HALO):
        if i < 0:
            i = -i
        elif i > N_WIN - 1:
            i = 2 * (N_WIN - 1) - i
        ids.append(i)
    ids.append(ids[-1])  # pad to 200
    return np.asarray(ids, dtype=np.int64)


def prep_inputs(x, W1, b1, W2, b2, conv_w):
    """Full inputs -> list of 8 per-core input dicts."""
    x = np.asarray(x, np.float32)
    W1 = np.asarray(W1, np.float32)
    b1 = np.asarray(b1, np.float32)
    W2 = np.asarray(W2, np.float32)
    b2 = np.asarray(b2, np.float32)
    conv_w = np.asarray(conv_w, np.float32)

    xs = ((x - 0.5) * 2.0).astype(BF16NP)
    # [b, win, ch, wl, c] -> [wl, win, ch, b, c]   (bc = b*2 + c interleaved)
    xr0 = xs.reshape(B, N_WIN, 4, 125, 2).transpose(3, 1, 2, 0, 4)
    xr = np.ones((126, N_WIN, 4, 64, 2), BF16NP)  # row 125 = bias ones
    xr[:125] = xr0

    w1p = np.zeros((N_WIN, WIN, HP), np.float32)
    w1p[:, :, :HID] = W1
    # [win, ch, wl, h] -> [wl, win, ch, h]; row 125 ch0 = b1
    w1r = np.zeros((126, N_WIN, 4, HP), np.float32)
    w1r[:125] = w1p.reshape(N_WIN, 4, 125, HP).transpose(2, 0, 1, 3)
    w1r[125, :, 0, :HID] = b1
    w1r = w1r.astype(BF16NP)

    # Toeplitz bands [yin, i, xt*378 + o*54 + yout]
    tpz = np.zeros((128, ANC, 756), np.float32)
    yo = np.arange(54)
    for xt in range(2):
        for o in range(ANC):
            s = xt * ANC + o
            for i in range(ANC):
                for t in range(KS):
                    tpz[yo + t, i, s * 54 + yo] = conv_w[o, i, t, xt]
    tpz = tpz.astype(BF16NP)

    # transpose permutations: out col m (c-major, m = c*64+b) reads input
    # partition b*2+c (prm[0]) or b*2+(1-c) (prm[1], channel-flipped)
    m = np.arange(128)
    bb, cc = m % 64, m // 64
    prm = np.zeros((128, 2, 128), np.float32)
    prm[bb * 2 + cc, 0, m] = 1.0
    prm[bb * 2 + (1 - cc), 1, m] = 1.0
    prm = prm.astype(BF16NP)

    in_maps = []
    for k in range(NCORES):
        ids = core_windows(k)
        xkb = np.zeros((25, 128, 8, 4, 128), BF16NP)
        xkb[:, :126] = xr[:, ids].reshape(126, 25, 8, 4, 128).transpose(
            1, 0, 2, 3, 4)
        w1kb = np.zeros((25, 128, 8, 4, HP), BF16NP)
        w1kb[:, :126] = w1r[:, ids].reshape(126, 25, 8, 4, HP).transpose(
            1, 0, 2, 3, 4)

        w2kb = np.zeros((128, 3, 504), np.float32)
        b2kb = np.zeros((128, 3, 504), np.float32)
        for n in range(LWP):
            g, kk = n // 4, n % 4
            j, s = g // 18, g % 18
            w2kb[32 * kk:32 * kk + HID, j, 28 * s + 7 * kk:28 * s + 7 * kk + 7] = \
                W2[ids[n]]
            b2kb[:, j, 28 * s + 7 * kk:28 * s + 7 * kk + 7] = b2[ids[n]][None, :]
        in_maps.append({
            "xk": xkb, "w1k": w1kb,
            "w2k": w2kb.astype(BF16NP), "b2k": b2kb, "tpzk": tpz,
            "prmk": prm,
        })
    return in_maps


def assemble(results, swap_combine=True):
    """Per-core {'ob','sm'} blobs -> full (out_base, out_smooth).

    ob partitions are bc = b*2 + c; sm partitions are c-major with cx
    flipped (conv swap trick)."""
    ob = np.empty((B, ANC, N_WIN, 2), np.float32)
    sm = np.empty((B, ANC, N_WIN, 2), np.float32)
    for k in range(NCORES):
        obk = results[k]["ob"].reshape(64, 2, 125, ANC).transpose(0, 3, 2, 1)
        ob[:, :, OWN * k:OWN * (k + 1), :] = obk
        smk = results[k]["sm"].reshape(2, 64, 125, ANC).transpose(1, 3, 2, 0)
        sm[:, :, OWN * k:OWN * (k + 1), :] = smk[:, :, :, ::-1]
    return ob, sm


def _run_device(in_maps, conv_b, trace=False):
    import concourse.bacc as bacc
    from concourse import bass_utils
    nc = bacc.Bacc("TRN2", target_bir_lowering=False, debug=False,
                   num_devices=NCORES)
    build(nc, conv_b)
    nc.compile()
    res = bass_utils.run_bass_kernel_spmd(
        nc, in_maps, core_ids=list(range(NCORES)), trace=trace)
    return res


def _kernel_numpy(x, W1, b1, W2, b2, conv_w, conv_b):
    x = (np.asarray(x, np.float32) - 0.5) * 2.0
    xw = x.reshape(B, N_WIN, WIN, 2)
    ob = np.empty((B, ANC, N_WIN, 2), np.float32)
    p = np.empty((B, ANC, N_WIN, 2), np.float32)
    for n0 in range(0, N_WIN, 50):
        sl = slice(n0, n0 + 50)
        h = np.einsum('bnwc,nwh->bnhc', xw[:, sl], W1[sl])
        h += b1[sl][None, :, :, None]
        np.maximum(h, 0.0, out=h)
        mean = h.mean(axis=0, keepdims=True)
        var = h.var(axis=0, keepdims=True)
        h = (h - mean) / np.sqrt(var + EPS)
        o = np.einsum('bnhc,nha->bnac', h, W2[sl]) + b2[sl][None, :, :, None]
        o = np.transpose(o, (0, 2, 1, 3))
        ob[:, :, sl] = o
        e = np.exp(o - o.max(axis=1, keepdims=True))
        p[:, :, sl] = e / e.sum(axis=1, keepdims=True)
    pp = np.pad(p, ((0, 0), (0, 0), (KS // 2, KS // 2), (1, 1)), mode='reflect')
    Bn, Ci, Hh, Ww = pp.shape
    sm = np.zeros((Bn, ANC, Hh - KS + 1, 2), np.float32)
    for t in range(KS):
        for w in range(2):
            sm += np.einsum('oi,biyx->boyx', conv_w[:, :, t, w],
                            pp[:, :, t:t + Hh - KS + 1, w:w + 2])
    sm += conv_b[None, :, None, None]
    return ob, sm


def kernel(x, W1, b1, W2, b2, conv_w, conv_b):
    W1 = np.asarray(W1, np.float32)
    b1 = np.asarray(b1, np.float32)
    W2 = np.asarray(W2, np.float32)
    b2 = np.asarray(b2, np.float32)
    conv_w = np.asarray(conv_w, np.float32)
    conv_b = np.asarray(conv_b, np.float32)
    try:
        in_maps = prep_inputs(x, W1, b1, W2, b2, conv_w)
        res = _run_device(in_maps, conv_b, trace=False)
        return assemble(res.results)
    except Exception as e:
        print(f"kernel: device path failed ({type(e).__name__}: {e}); "
              "falling back to numpy", file=sys.stderr)
        return _kernel_numpy(x, W1, b1, W2, b2, conv_w, conv_b)


# revision 3
# speedup vs baseline: 1.0224x; 1.0224x over previous
"""LAINet forward (nn_LAINetOriginal) as a Bass/Tile kernel on 8 trn2 cores.

Window sharding 8 x 125 (+37-window reflect halo recomputed per core so the
conv smoother is core-local). Per core: bf16 matmul1 with b1 folded as an
extra contraction row, fused ReLU copy, batch-norm via single bn_stats per
4-window group, block-diagonal W2 matmul2, free-axis softmax, PE transposes
with host-built permutation gathers, conv as Toeplitz-band matmuls with
shared-psum channel-reflect accumulation.

Self-contained: host shards full inputs, compiles + runs the SPMD NEFF via
concourse/bass (system toolchain), gathers full outputs. Numpy fallback if
the device path is unavailable.
"""
import sys

for _p in ('/opt/trn_rl_repo',):
    if _p not in sys.path:
        sys.path.insert(0, _p)

"""LAINet forward on trn2 — Bass/Tile kernel builder + host shard prep.

Sharding: 1000 windows split 8 x 125; each core recomputes a 37-window
reflect-mapped halo so the conv smoother is fully core-local.

Per-core layout (bc = b*2 + c, channel-interleaved):
  matmul1: per window n (200 = 125 own + 74 halo + 1 pad), 4 K-chunks of 125:
    lhsT = W1 chunk [125, 32(h padded)], rhs = x chunk [125, 128(bc)]
    -> psum1 [128, 128], 4 windows packed at 32-partition strides.
  relu+b1 fused in ACT copy psum->sbuf (bf16).
  BN over b: single bn_stats per group (even/odd lanes = c0/c1), variance
    read straight from the count*var slot with scale=1/64; apply on GpSimd.
  matmul2: lhsT = h_norm group [128, 128], rhs = block-diag W2 [128, 28]
    -> psum2 [128, bank, slot*28] ([bc, n, a] layout).
  softmax over a on the free axis; p -> PE transposes -> [win, a, bc].
  conv: Toeplitz-band matmuls, lhsT = pT/pS (pair-swapped) with a c-major
    column reorder so slot o accumulates both x-taps in shared psum
    partitions: m<64 -> out[cx=1], m>=64 -> out[cx=0] (host flips cx).
"""
"""LAINet forward on trn2 — Bass/Tile kernel builder + host shard prep.

Sharding: 1000 windows split 8 x 125; each core recomputes a 37-window
reflect-mapped halo so the conv smoother is fully core-local.

Per-core layout (bc = b*2 + c, channel-interleaved):
  matmul1: per window n (200 = 125 own + 74 halo + 1 pad), 4 K-chunks of 125:
    lhsT = W1 chunk [125, 32(h padded)], rhs = x chunk [125, 128(bc)]
    -> psum1 [128, 128], 4 windows packed at 32-partition strides.
  relu+b1 fused in ACT copy psum->sbuf (bf16).
  BN over b: single bn_stats per group (even/odd lanes = c0/c1), variance
    read straight from the count*var slot with scale=1/64; apply on GpSimd.
  matmul2: lhsT = h_norm group [128, 128], rhs = block-diag W2 [128, 28]
    -> psum2 [128, bank, slot*28] ([bc, n, a] layout).
  softmax over a on the free axis; p -> PE transposes -> [win, a, bc].
  conv: Toeplitz-band matmuls, lhsT = pT/pS (pair-swapped) with a c-major
    column reorder so slot o accumulates both x-taps in shared psum
    partitions: m<64 -> out[cx=1], m>=64 -> out[cx=0] (host flips cx).
"""
import numpy as np
import ml_dtypes

BF16NP = ml_dtypes.bfloat16
FP8NP = ml_dtypes.float8_e4m3
W1SCALE = 16.0

B = 64
INPUT_DIM = 500000
WIN = 500
N_WIN = 1000
HID = 30
HP = 32          # padded hidden
ANC = 7
KS = 75
EPS = 1e-5
NCORES = 8
OWN = 125
HALO = 37
LW = 199
LWP = 200        # padded local window count
NG = 50          # groups of 4 windows
GPB = (18, 18, 14)   # groups per psum2 bank
NWB = (72, 72, 56)   # windows per bank
Y0S = (0, 54, 108)   # conv block output offsets
RS = (128, 128, 91)  # conv block input row counts
NOUTS = (54, 54, 17) # conv block output counts
GBAT = 5             # groups per DMA batch
NBAT = NG // GBAT    # 10


def build(nc, conv_b, swap_combine=True):
    """Construct the SPMD per-core program. conv_b: np [7] baked as imms."""
    import concourse.tile as tile
    import concourse.mybir as mybir
    from contextlib import ExitStack

    F32 = mybir.dt.float32
    BF = mybir.dt.bfloat16
    AF = mybir.ActivationFunctionType
    OP = mybir.AluOpType
    AX = mybir.AxisListType

    xk = nc.dram_tensor("xk", [25, 128, 8, 4, 128], BF,
                        kind="ExternalInput")
    w1k = nc.dram_tensor("w1k", [25, 128, 8, 4, HP], BF,
                         kind="ExternalInput")
    w2k = nc.dram_tensor("w2k", [128, 3, 504], BF, kind="ExternalInput")
    b2k = nc.dram_tensor("b2k", [128, 3, 504], F32, kind="ExternalInput")
    tpzk = nc.dram_tensor("tpzk", [128, ANC, 756], BF, kind="ExternalInput")
    prmk = nc.dram_tensor("prmk", [128, 2, 128], BF, kind="ExternalInput")
    ob_d = nc.dram_tensor("ob", [128, 875], F32, kind="ExternalOutput")
    sm_d = nc.dram_tensor("sm", [128, 875], F32, kind="ExternalOutput")

    with tile.TileContext(nc) as tc, ExitStack() as ctx:
        const = ctx.enter_context(tc.tile_pool(name="const", bufs=1))
        o_sb = const.tile([128, 3, 504], F32)
        e_sb = const.tile([128, 3, 504], BF)
        p_sb = const.tile([128, LWP, ANC], BF)
        sm_sb = const.tile([128, 125, ANC], F32)

        xp = ctx.enter_context(tc.tile_pool(name="xp", bufs=8))
        wp = ctx.enter_context(tc.tile_pool(name="wp", bufs=8))
        hpool = ctx.enter_context(tc.tile_pool(name="hp", bufs=6))
        sp = ctx.enter_context(tc.tile_pool(name="sp", bufs=3))
        ptp = ctx.enter_context(tc.tile_pool(name="ptp", bufs=2))
        ps1 = ctx.enter_context(tc.tile_pool(name="ps1", bufs=2, space="PSUM"))
        ps2 = ctx.enter_context(tc.tile_pool(name="ps2", bufs=2, space="PSUM"))
        tpq = ctx.enter_context(tc.tile_pool(name="tpq", bufs=3, space="PSUM"))
        pcp = ctx.enter_context(tc.tile_pool(name="pcp", bufs=1, space="PSUM"))

        # pair-sized (2-group) transfers; x on the sync HWDGE ring,
        # w1 via gpsimd SWDGE, consts on the scalar ring.
        xts = []
        wts = []
        for p in range(25):
            xt = xp.tile([128, 8, 4, 128], BF, tag="xb", name=f"xt{p}")
            if p == 0:
                nc.sync.dma_start(out=xt[:, 0:4], in_=xk.ap()[0][:, 0:4])
                nc.sync.dma_start(out=xt[:, 4:8], in_=xk.ap()[0][:, 4:8])
            else:
                nc.sync.dma_start(out=xt, in_=xk.ap()[p])
            wt = wp.tile([128, 8, 4, HP], BF, tag="wb", name=f"wt{p}")
            nc.gpsimd.dma_start(out=wt, in_=w1k.ap()[p])
            xts.append(xt)
            wts.append(wt)
            if p == 0:
                w2_sb = const.tile([128, 3, 504], BF)
                nc.scalar.dma_start(out=w2_sb, in_=w2k.ap())
                b2_sb = const.tile([128, 3, 504], F32)
                nc.scalar.dma_start(out=b2_sb, in_=b2k.ap())
                tz_sb = const.tile([128, ANC, 756], BF)
                nc.scalar.dma_start(out=tz_sb, in_=tpzk.ap())
                prm_sb = const.tile([128, 2, 128], BF)
                nc.scalar.dma_start(out=prm_sb, in_=prmk.ap())
                eps_sb = const.tile([128, 1], F32)
                nc.vector.memset(eps_sb, EPS)

        psum2 = {}

        def softmax_bank(j):
            nwj = NWB[j]
            cols = nwj * ANC
            nc.vector.tensor_tensor(
                out=o_sb[:, j, 0:cols], in0=psum2[j][:, 0:cols],
                in1=b2_sb[:, j, 0:cols], op=OP.add)
            nc.scalar.activation(out=e_sb[:, j, 0:cols],
                                 in_=o_sb[:, j, 0:cols], func=AF.Exp)
            ev = e_sb[:, j, 0:cols].rearrange("p (w a) -> p w a", a=ANC)
            rsum = sp.tile([128, 72], F32, tag="rsum", name=f"rsum{j}")
            nc.vector.reduce_sum(out=rsum[:, 0:nwj], in_=ev, axis=AX.X)
            nc.vector.reciprocal(out=rsum[:, 0:nwj], in_=rsum[:, 0:nwj])
            rb = rsum[:, 0:nwj].broadcast_to([128, nwj, ANC])
            nc.vector.tensor_tensor(
                out=p_sb[:, 72 * j:72 * j + nwj, :], in0=ev, in1=rb,
                op=OP.mult)
            # out_base slice for this bank (own windows 37..161)
            lo, hi = max(0, 37 - 72 * j), min(nwj, 162 - 72 * j)
            off = (72 * j + lo) - 37
            nc.gpsimd.dma_start(
                out=ob_d.ap()[:, 7 * off:7 * (off + hi - lo)],
                in_=o_sb[:, j, 7 * lo:7 * hi])

        def conv_block(t):
            y0, r, nout = Y0S[t], RS[t], NOUTS[t]
            pT = ptp.tile([128, ANC, 128], BF, tag="pT", name=f"pT{t}")
            pS = ptp.tile([128, ANC, 128], BF, tag="pS", name=f"pS{t}")
            for a2 in range(ANC):
                tp = tpq.tile([128, 128], BF, tag="tpq", name=f"tp{t}_{a2}")
                nc.tensor.transpose(out=tp[0:r, :], in_=p_sb[:, y0:y0 + r, a2],
                                    identity=prm_sb[:, 0, :])
                if a2 % 2 == 0:
                    nc.vector.tensor_copy(out=pT[0:r, a2, :], in_=tp[0:r, :])
                else:
                    nc.scalar.copy(out=pT[0:r, a2, :], in_=tp[0:r, :])
                tq = tpq.tile([128, 128], BF, tag="tpq", name=f"tq{t}_{a2}")
                nc.tensor.transpose(out=tq[0:r, :], in_=p_sb[:, y0:y0 + r, a2],
                                    identity=prm_sb[:, 1, :])
                if a2 % 2 == 0:
                    nc.scalar.copy(out=pS[0:r, a2, :], in_=tq[0:r, :])
                else:
                    nc.vector.tensor_copy(out=pS[0:r, a2, :], in_=tq[0:r, :])
            psc = pcp.tile([128, 378], F32, tag="psc", name=f"psc{t}")
            for i in range(ANC):
                nc.tensor.matmul(psc, lhsT=pT[0:r, i, :],
                                 rhs=tz_sb[0:r, i, 0:378],
                                 start=(i == 0), stop=False)
            for i in range(ANC):
                nc.tensor.matmul(psc, lhsT=pS[0:r, i, :],
                                 rhs=tz_sb[0:r, i, 378:756],
                                 start=False, stop=(i == 6))
            for o in range(ANC):
                nc.vector.tensor_scalar_add(
                    out=sm_sb[:, y0:y0 + nout, o],
                    in0=psc[:, 54 * o:54 * o + nout],
                    scalar1=float(conv_b[o]))
            nc.gpsimd.dma_start(
                out=sm_d.ap()[:, 7 * y0:7 * (y0 + nout)],
                in_=sm_sb[:, y0:y0 + nout, :])


        # pair compute: mm1 x32 -> relu -> stats; batch (5-group) sqrt,
        # then applies + mm2 per group; softmax as each bank completes.
        sts = {}
        hgs = {}
        for p in range(25):
            pt = ps1.tile([128, 256], F32, tag="pt", name=f"pt{p}")
            for sl in range(2):
                g = 2 * p + sl
                for k in range(4):
                    for ch in range(4):
                        kk = 126 if ch == 0 else 125
                        nc.tensor.matmul(
                            pt[32 * k:32 * k + HP, 128 * sl:128 * sl + 128],
                            lhsT=wts[p][0:kk, 4 * sl + k, ch, :],
                            rhs=xts[p][0:kk, 4 * sl + k, ch, :],
                            start=(ch == 0), stop=(ch == 3),
                            tile_position=(0, 32 * k))
            hg = hpool.tile([128, 256], BF, tag="hg", name=f"hg{p}")
            nc.scalar.activation(
                out=hg, in_=pt, func=AF.Relu, bias=0.0, scale=1.0)
            for sl in range(2):
                g = 2 * p + sl
                ib = g // GBAT
                if ib not in sts:
                    sts[ib] = sp.tile([128, GBAT, 6], F32, tag="st",
                                      name=f"st{ib}")
                hgs[g] = (hg, sl)
                nc.vector.bn_stats(out=sts[ib][:, g % GBAT, :],
                                   in_=hg[:, 128 * sl:128 * sl + 128])
                if g % GBAT != GBAT - 1:
                    continue
                st = sts[ib]
                stv = st.rearrange("p g (c s) -> p g c s", s=3)
                rs = sp.tile([128, GBAT, 2], F32, tag="rs", name=f"rs{ib}")
                nc.scalar.activation(
                    out=rs, in_=stv[:, :, :, 2], func=AF.Sqrt,
                    bias=eps_sb[:, 0:1], scale=1.0 / 64.0)
                nc.vector.reciprocal(out=rs, in_=rs)
                for gg in range(GBAT):
                    g2 = GBAT * ib + gg
                    hg2, sl2 = hgs[g2]
                    hgi = hg2.rearrange("p (s b c) -> p s b c", b=64, c=2)
                    for c in range(2):
                        nc.vector.tensor_scalar(
                            out=hgi[:, sl2, :, c], in0=hgi[:, sl2, :, c],
                            scalar1=stv[:, gg, c, 1:2],
                            scalar2=rs[:, gg, c:c + 1],
                            op0=OP.subtract, op1=OP.mult)
                    j = g2 // 18
                    sj = g2 - 18 * j
                    if j not in psum2:
                        psum2[j] = ps2.tile([128, 512], F32, tag="ps2",
                                            name=f"psum2_{j}")
                    nc.tensor.matmul(
                        psum2[j][:, 28 * sj:28 * sj + 28],
                        lhsT=hg2[:, 128 * sl2:128 * sl2 + 128],
                        rhs=w2_sb[:, j, 28 * sj:28 * sj + 28],
                        start=True, stop=True)
                for j in range(3):
                    if min(18 * j + 17, NG - 1) // GBAT == ib:
                        softmax_bank(j)

        # conv: transposes + Toeplitz matmuls with shared-slot accumulation.
        # A-branch (xt=0) uses pT (c-major gather); B-branch (xt=1) uses the
        # channel-flipped gather pS: slot o partition m holds out[cx=1] for
        # m<64 and out[cx=0] for m>=64 (host flips cx).
        for t in range(len(Y0S)):
            conv_block(t)
        return nc


def core_windows(k):
    ids = []
    for i in range(OWN * k - HALO, OWN * (k + 1) + HALO):
        if i < 0:
            i = -i
        elif i > N_WIN - 1:
            i = 2 * (N_WIN - 1) - i
        ids.append(i)
    ids.append(ids[-1])  # pad to 200
    return np.asarray(ids, dtype=np.int64)


def prep_inputs(x, W1, b1, W2, b2, conv_w):
    """Full inputs -> list of 8 per-core input dicts."""
    x = np.asarray(x, np.float32)
    W1 = np.asarray(W1, np.float32)
    b1 = np.asarray(b1, np.float32)
    W2 = np.asarray(W2, np.float32)
    b2 = np.asarray(b2, np.float32)
    conv_w = np.asarray(conv_w, np.float32)

    xs = ((x - 0.5) * 2.0).astype(BF16NP)
    # [b, win, ch, wl, c] -> [wl, win, ch, b, c]   (bc = b*2 + c interleaved)
    xr0 = xs.reshape(B, N_WIN, 4, 125, 2).transpose(3, 1, 2, 0, 4)
    xr = np.ones((126, N_WIN, 4, 64, 2), BF16NP)  # row 125 = bias ones
    xr[:125] = xr0

    w1p = np.zeros((N_WIN, WIN, HP), np.float32)
    w1p[:, :, :HID] = W1
    # [win, ch, wl, h] -> [wl, win, ch, h]; row 125 ch0 = b1
    w1r = np.zeros((126, N_WIN, 4, HP), np.float32)
    w1r[:125] = w1p.reshape(N_WIN, 4, 125, HP).transpose(2, 0, 1, 3)
    w1r[125, :, 0, :HID] = b1
    w1r = w1r.astype(BF16NP)

    # Toeplitz bands [yin, i, xt*378 + o*54 + yout]
    tpz = np.zeros((128, ANC, 756), np.float32)
    yo = np.arange(54)
    for xt in range(2):
        for o in range(ANC):
            s = xt * ANC + o
            for i in range(ANC):
                for t in range(KS):
                    tpz[yo + t, i, s * 54 + yo] = conv_w[o, i, t, xt]
    tpz = tpz.astype(BF16NP)

    # transpose permutations: out col m (c-major, m = c*64+b) reads input
    # partition b*2+c (prm[0]) or b*2+(1-c) (prm[1], channel-flipped)
    m = np.arange(128)
    bb, cc = m % 64, m // 64
    prm = np.zeros((128, 2, 128), np.float32)
    prm[bb * 2 + cc, 0, m] = 1.0
    prm[bb * 2 + (1 - cc), 1, m] = 1.0
    prm = prm.astype(BF16NP)

    in_maps = []
    for k in range(NCORES):
        ids = core_windows(k)
        xkb = np.zeros((25, 128, 8, 4, 128), BF16NP)
        xkb[:, :126] = xr[:, ids].reshape(126, 25, 8, 4, 128).transpose(
            1, 0, 2, 3, 4)
        w1kb = np.zeros((25, 128, 8, 4, HP), BF16NP)
        w1kb[:, :126] = w1r[:, ids].reshape(126, 25, 8, 4, HP).transpose(
            1, 0, 2, 3, 4)

        w2kb = np.zeros((128, 3, 504), np.float32)
        b2kb = np.zeros((128, 3, 504), np.float32)
        for n in range(LWP):
            g, kk = n // 4, n % 4
            j, s = g // 18, g % 18
            w2kb[32 * kk:32 * kk + HID, j, 28 * s + 7 * kk:28 * s + 7 * kk + 7] = \
                W2[ids[n]]
            b2kb[:, j, 28 * s + 7 * kk:28 * s + 7 * kk + 7] = b2[ids[n]][None, :]
        in_maps.append({
            "xk": xkb, "w1k": w1kb,
            "w2k": w2kb.astype(BF16NP), "b2k": b2kb, "tpzk": tpz,
            "prmk": prm,
        })
    return in_maps


def assemble(results, swap_combine=True):
    """Per-core {'ob','sm'} blobs -> full (out_base, out_smooth).

    ob partitions are bc = b*2 + c; sm partitions are c-major with cx
    flipped (conv swap trick)."""
    ob = np.empty((B, ANC, N_WIN, 2), np.float32)
    sm = np.empty((B, ANC, N_WIN, 2), np.float32)
    for k in range(NCORES):
        obk = results[k]["ob"].reshape(64, 2, 125, ANC).transpose(0, 3, 2, 1)
        ob[:, :, OWN * k:OWN * (k + 1), :] = obk
        smk = results[k]["sm"].reshape(2, 64, 125, ANC).transpose(1, 3, 2, 0)
        sm[:, :, OWN * k:OWN * (k + 1), :] = smk[:, :, :, ::-1]
    return ob, sm


def _run_device(in_maps, conv_b, trace=False):
    import concourse.bacc as bacc
    from concourse import bass_utils
    nc = bacc.Bacc("TRN2", target_bir_lowering=False, debug=False,
                   num_devices=NCORES)
    build(nc, conv_b)
    nc.compile()
    res = bass_utils.run_bass_kernel_spmd(
        nc, in_maps, core_ids=list(range(NCORES)), trace=trace)
    return res


def _kernel_numpy(x, W1, b1, W2, b2, conv_w, conv_b):
    x = (np.asarray(x, np.float32) - 0.5) * 2.0
    xw = x.reshape(B, N_WIN, WIN, 2)
    ob = np.empty((B, ANC, N_WIN, 2), np.float32)
    p = np.empty((B, ANC, N_WIN, 2), np.float32)
    for n0 in range(0, N_WIN, 50):
        sl = slice(n0, n0 + 50)
        h = np.einsum('bnwc,nwh->bnhc', xw[:, sl], W1[sl])
        h += b1[sl][None, :, :, None]
        np.maximum(h, 0.0, out=h)
        mean = h.mean(axis=0, keepdims=True)
        var = h.var(axis=0, keepdims=True)
        h = (h - mean) / np.sqrt(var + EPS)
        o = np.einsum('bnhc,nha->bnac', h, W2[sl]) + b2[sl][None, :, :, None]
        o = np.transpose(o, (0, 2, 1, 3))
        ob[:, :, sl] = o
        e = np.exp(o - o.max(axis=1, keepdims=True))
        p[:, :, sl] = e / e.sum(axis=1, keepdims=True)
    pp = np.pad(p, ((0, 0), (0, 0), (KS // 2, KS // 2), (1, 1)), mode='reflect')
    Bn, Ci, Hh, Ww = pp.shape
    sm = np.zeros((Bn, ANC, Hh - KS + 1, 2), np.float32)
    for t in range(KS):
        for w in range(2):
            sm += np.einsum('oi,biyx->boyx', conv_w[:, :, t, w],
                            pp[:, :, t:t + Hh - KS + 1, w:w + 2])
    sm += conv_b[None, :, None, None]
    return ob, sm


def kernel(x, W1, b1, W2, b2, conv_w, conv_b):
    W1 = np.asarray(W1, np.float32)
    b1 = np.asarray(b1, np.float32)
    W2 = np.asarray(W2, np.float32)
    b2 = np.asarray(b2, np.float32)
    conv_w = np.asarray(conv_w, np.float32)
    conv_b = np.asarray(conv_b, np.float32)
    try:
        in_maps = prep_inputs(x, W1, b1, W2, b2, conv_w)
        res = _run_device(in_maps, conv_b, trace=False)
        return assemble(res.results)
    except Exception as e:
        print(f"kernel: device path failed ({type(e).__name__}: {e}); "
              "falling back to numpy", file=sys.stderr)
        return _kernel_numpy(x, W1, b1, W2, b2, conv_w, conv_b)
